# revision 5
# baseline (speedup 1.0000x reference)
"""Multi-head attention (B=2, S=2048, H=1024, 16 heads) on 8 TRN2 NeuronCores.

Sharding: core c -> batch b = c//4, head-group g = c%4 (heads 4g..4g+3).
Each core computes q/k/v projections for its 4 heads (tensor parallel),
full attention for those heads, and a partial output projection
(contribution of its 256 hidden dims). Host sums the 4 partials per batch
(bf16 partials, fp32 sum) and adds the output bias.

The kernel is jointly ACT(exp)- and PE-limited (~150-165us each). The
emission is built around a continuous scores->exp stream:

  - Weights are host-packed so each tensor is ONE contiguous DMA (DMA issue
    costs ~650ns each, serialized per queue). xt is DMA'd in column halves:
    cols 0:1024 on the Sync HWDGE queue, cols 1024:2048 on the Scalar HWDGE
    queue (idle until the exp stream starts), so the first k/q projection
    groups can finish ~8us in.
  - One "slot" per (pair, ib, jj): emits the two row-packed K=64 score
    matmuls (concurrent on the PE via row groups, start delta ~4ns) + the
    exp ACTIVATE, then pumps a work queue of deferrable units: v-proj
    chunks, qk projection half-groups (N=256, atomic psum lifetime), PV
    matmuls (lagging the exp stream), softmax epilogues and the output
    projection. Deadline-critical kt/qt projections are pinned to fixed
    slots. The pump budget adapts to the remaining queue so work cannot
    pile up into a serial tail; a force-drain rule keeps the PV lag under
    the ex ring depth so the statically-ordered engine queues cannot
    deadlock.
  - softmax epilogue: row-sum l comes free from a ones-column appended to V
    (PV matmul M=65); 1/l = exp(-ln(l)) on ACT, batched per (pair, ib) as
    one [1,1024] ln + one [1,1024] exp; broadcast to 64 partitions via a
    K=1 matmul; normalize on DVE. (reciprocal_approx_fast and gpsimd
    partition_broadcast both return garbage on this environment's HW --
    custom-DVE/gpsimd tables are not loaded -- so only ACT-table math and
    plain matmul/DVE ops are used.)
  - out-proj psum is CAST to bf16 and DMA'd as bf16 partials (halves the
    output traffic; host sums in fp32).
"""

import os
from collections import deque
from contextlib import ExitStack

import numpy as np
import ml_dtypes

B = 2
S = 2048
HID = 1024
NHEAD = 16
HDIM = 64
NCORES = 8
GROUPS = 4  # head-groups per batch (cores per batch)
DH = 256  # hidden dims per core (4 heads x 64)
SCALE = 1.0 / np.sqrt(np.float32(HDIM))  # 0.125

EXBUFS = 14  # ex ring depth; PV may lag the exp stream by at most EXBUFS-2

_CACHE = {}
last_exec_time_ns = None
last_results = None


def _build_graph(with_qkv_bias: bool):
    import concourse.bass as bass
    import concourse.mybir as mybir
    import concourse.tile as tile
    from concourse import bacc

    F32 = mybir.dt.float32
    BF16 = mybir.dt.bfloat16
    EXP = mybir.ActivationFunctionType.Exp
    LN = mybir.ActivationFunctionType.Ln

    # The kernel uses both Exp and Ln. Left alone, the act-table-load pass
    # alternates between exp_and_others and natural_log (17 loads, ~2.7us
    # each). Steer it to the one set containing both by hiding Exp/Ln from
    # every other set (indices must stay stable, so entries are kept).
    if not getattr(bacc, "_mha_act_tabs_patched", False):
        orig_gat = bacc.get_activation_tables

        def _gat(arch, _orig=orig_gat):
            out = {}
            for n, s in _orig(arch).items():
                if n != "natural_log_exp_and_others":
                    s = s - {EXP, LN}
                out[n] = s
            return out

        bacc.get_activation_tables = _gat
        bacc._mha_act_tabs_patched = True

    nc = bacc.Bacc()
    # weights host-packed e-major so each is one contiguous DMA
    xt_d = nc.declare_dram_parameter("xt", [HID, S], BF16, isOutput=False)
    wq_d = nc.declare_dram_parameter("wq", [128, 8 * DH], BF16, isOutput=False)
    wk_d = nc.declare_dram_parameter("wk", [128, 8 * DH], BF16, isOutput=False)
    wv_d = nc.declare_dram_parameter("wv", [128, 8 * DH], BF16, isOutput=False)
    wo_d = nc.declare_dram_parameter("wo", [128, 2 * HID], BF16, isOutput=False)
    if with_qkv_bias:
        bq_d = nc.declare_dram_parameter("bq", [1, DH], BF16, isOutput=False)
        bk_d = nc.declare_dram_parameter("bk", [1, DH], BF16, isOutput=False)
        bv_d = nc.declare_dram_parameter("bv", [1, DH], BF16, isOutput=False)
    out_d = nc.declare_dram_parameter("out", [S, HID], BF16, isOutput=True)

    with ExitStack() as ctx:
        tc = ctx.enter_context(tile.TileContext(nc))
        cons = ctx.enter_context(tc.tile_pool(name="cons", bufs=1))
        work = ctx.enter_context(tc.tile_pool(name="work", bufs=3))
        scp = ctx.enter_context(tc.tile_pool(name="scp", bufs=2, space="PSUM"))
        pvp = ctx.enter_context(tc.tile_pool(name="pvp", bufs=1, space="PSUM"))
        mip = ctx.enter_context(tc.tile_pool(name="mip", bufs=2, space="PSUM"))

        # ---- SBUF tiles -------------------------------------------------
        xt_sb = [
            cons.tile([128, S], BF16, name=f"xt{e}", tag=f"xt{e}") for e in range(8)
        ]
        wq_sb = cons.tile([128, 8, DH], BF16, name="wq", tag="wq")
        wk_sb = cons.tile([128, 8, DH], BF16, name="wk", tag="wk")
        wv_sb = cons.tile([128, 8, DH], BF16, name="wv", tag="wv")
        wo_sb = cons.tile([128, 2, HID], BF16, name="wo", tag="wo")

        # DMA order: many small col-block DMAs stream at ~650ns issue each
        # with the 16 transfer engines running behind; big single DMAs
        # serialize. Dependency order across the two HWDGE queues:
        #   Sync:   wk, xt-c0 (e=0..7), xt-c2 (e=0..7)
        #   Scalar: wq, wv, xt-c1, xt-c3, wo
        # so k0.sb0/q0.sb0 complete ~14us in and kt col-blocks stay ahead of
        # the exp stream.
        def dma_xt_col(eng, e, sb):
            eng.dma_start(
                out=xt_sb[e][:, sb * 512 : (sb + 1) * 512],
                in_=xt_d[e * 128 : (e + 1) * 128, sb * 512 : (sb + 1) * 512],
            )

        nc.sync.dma_start(out=wk_sb, in_=wk_d[:, :])
        nc.scalar.dma_start(out=wq_sb, in_=wq_d[:, :])
        nc.scalar.dma_start(out=wv_sb, in_=wv_d[:, :])
        for e in range(8):
            dma_xt_col(nc.sync, e, 0)
        for e in range(8):
            dma_xt_col(nc.scalar, e, 1)
        for e in range(8):
            dma_xt_col(nc.sync, e, 2)
        for e in range(8):
            dma_xt_col(nc.scalar, e, 3)
        nc.scalar.dma_start(out=wo_sb, in_=wo_d[:, :])

        ones1 = cons.tile([1, 512], BF16, name="ones1", tag="ones1")
        nc.vector.memset(ones1, 1.0)
        # ones row at partition 64: stationary operand of the K=1 broadcast
        # matmul replicating rl16 to 64 partitions.
        ones64 = cons.tile([65, 64], BF16, name="ones64", tag="ones64")
        nc.vector.memset(ones64[64:65, :], 1.0)

        if with_qkv_bias:
            bias_sb = {}
            for nm, d in (("bq", bq_d), ("bk", bk_d), ("bv", bv_d)):
                t = cons.tile([1, DH], BF16, name=f"{nm}s", tag=f"{nm}s")
                nc.sync.dma_start(out=t, in_=d[:, :])
                bias_sb[nm] = t

        qt_sb = [
            cons.tile([128, S], BF16, name=f"qt{c}", tag=f"qt{c}") for c in range(2)
        ]
        kt_sb = [
            cons.tile([128, S], BF16, name=f"kt{c}", tag=f"kt{c}") for c in range(2)
        ]
        v_sb = [
            cons.tile([128, 4, 65], BF16, name=f"v{j}", tag=f"v{j}") for j in range(16)
        ]
        ctxn_sb = [
            [
                cons.tile([128, 512], BF16, name=f"cx{c}_{i}", tag=f"cx{c}_{i}")
                for i in range(4)
            ]
            for c in range(2)
        ]

        # ---- unit emitters ----------------------------------------------
        def qk_half(dst_sb, w_sb, bias_nm, cc, sb, half):
            # half-width projection group: atomic psum lifetime (N=256)
            ps = mip.tile([128, 256], F32, name=f"pq{cc}{sb}{half}", tag="mm")
            c0 = sb * 512 + half * 256
            for e in range(8):
                nc.tensor.matmul(
                    ps,
                    lhsT=w_sb[:, e, cc * 128 : (cc + 1) * 128],
                    rhs=xt_sb[e][:, c0 : c0 + 256],
                    start=(e == 0),
                    stop=(e == 7 and not with_qkv_bias),
                )
            if with_qkv_bias:
                nc.tensor.matmul(
                    ps,
                    lhsT=bias_sb[bias_nm][:, cc * 128 : (cc + 1) * 128],
                    rhs=ones1[:, 0:256],
                    start=False,
                    stop=True,
                )
            nc.vector.tensor_copy(out=dst_sb[cc][:, c0 : c0 + 256], in_=ps)

        def v_unit(jj):
            # v [s, d] natural, stored per j-chunk as [128, 4, 65] with a
            # ones column at [:, :, 64] for the softmax row-sum.
            ps = mip.tile([128, DH], F32, name=f"pv{jj}", tag="mm")
            for e in range(8):
                nc.tensor.matmul(
                    ps,
                    lhsT=xt_sb[e][:, jj * 128 : (jj + 1) * 128],
                    rhs=wv_sb[:, e, :],
                    start=(e == 0),
                    stop=(e == 7 and not with_qkv_bias),
                )
            if with_qkv_bias:
                nc.tensor.matmul(
                    ps,
                    lhsT=ones1[:, 0:128],
                    rhs=bias_sb["bv"],
                    start=False,
                    stop=True,
                )
            nc.vector.tensor_copy(
                out=v_sb[jj][:, :, 0:64],
                in_=ps.rearrange("p (h d) -> p h d", h=4),
            )
            nc.vector.memset(v_sb[jj][:, :, 64:65], 1.0)

        ex_ring = {}  # attention-stream index -> ex tile
        pv_ps = {}  # (pair, ib) -> [psum tiles h0, h1]
        epi_sb = {}  # (pair, ib) -> (pvsw, rl16)

        def sc_exp(pair, ib, jj, n):
            ps = scp.tile([128, 1024], F32, name=f"sc{pair}{ib}{jj}", tag="sc")
            for h in range(2):
                nc.tensor.matmul(
                    ps[:, h * 512 : (h + 1) * 512],
                    lhsT=kt_sb[pair][
                        h * 64 : (h + 1) * 64, jj * 128 : (jj + 1) * 128
                    ],
                    rhs=qt_sb[pair][
                        h * 64 : (h + 1) * 64, ib * 512 : (ib + 1) * 512
                    ],
                    start=True,
                    stop=True,
                )
            ex = work.tile(
                [128, 1024], BF16, name=f"ex{n}", tag="ex", bufs=EXBUFS
            )
            nc.scalar.activation(out=ex, in_=ps, func=EXP, scale=float(SCALE))
            ex_ring[n] = ex

        def pv_unit(pair, ib, jj, n):
            if jj == 0:
                pv_ps[(pair, ib)] = [
                    pvp.tile([65, 512], F32, name=f"pva{pair}{ib}", tag="pva"),
                    pvp.tile([65, 512], F32, name=f"pvb{pair}{ib}", tag="pvb"),
                ]
            ex = ex_ring.pop(n)
            for h in range(2):
                nc.tensor.matmul(
                    pv_ps[(pair, ib)][h],
                    lhsT=v_sb[jj][:, pair * 2 + h, :],
                    rhs=ex[:, h * 512 : (h + 1) * 512],
                    start=(jj == 0),
                    stop=(jj == 15),
                )

        def epi(pair, ib):
            # psum -> sbuf copies free the PV banks; one wide ln + one wide
            # exp on ACT give rl16 = 1/l for both heads in 2.3us.
            pvsw = work.tile(
                [65, 1024], F32, name=f"ps{pair}{ib}", tag="pvs", bufs=4
            )
            for h in range(2):
                nc.vector.tensor_copy(
                    out=pvsw[:, h * 512 : (h + 1) * 512],
                    in_=pv_ps[(pair, ib)][h],
                )
            lnl = work.tile([65, 1024], F32, name=f"ln{pair}{ib}", tag="lnl")
            nc.scalar.activation(out=lnl[64:65, :], in_=pvsw[64:65, :], func=LN)
            rl16 = work.tile(
                [65, 1024], BF16, name=f"rl{pair}{ib}", tag="rl", bufs=4
            )
            nc.scalar.activation(
                out=rl16[64:65, :], in_=lnl[64:65, :], func=EXP, scale=-1.0
            )
            del pv_ps[(pair, ib)]
            epi_sb[(pair, ib)] = (pvsw, rl16)

        def part2(pair, ib):
            pvsw, rl16 = epi_sb.pop((pair, ib))
            for h in range(2):
                bc = mip.tile([64, 512], F32, name=f"bc{pair}{ib}{h}", tag="mm")
                nc.tensor.matmul(
                    bc,
                    lhsT=ones64[64:65, :],
                    rhs=rl16[64:65, h * 512 : (h + 1) * 512],
                    start=True,
                    stop=True,
                )
                # DVE may read only one PSUM operand: in0 SBUF, in1 PSUM.
                if h == 0:
                    nc.vector.tensor_mul(
                        out=ctxn_sb[pair][ib][0:64, :],
                        in0=pvsw[0:64, 0:512],
                        in1=bc,
                    )
                else:
                    tmp = work.tile(
                        [64, 512], BF16, name=f"tmp{pair}{ib}", tag="tmp"
                    )
                    nc.vector.tensor_mul(
                        out=tmp, in0=pvsw[0:64, 512:1024], in1=bc
                    )
                    nc.sync.dma_start(out=ctxn_sb[pair][ib][64:128, :], in_=tmp)
            if pair == 1:
                for ss in range(4):
                    for eb in range(2):
                        enq(700, lambda ss=ss, eb=eb: po_unit(ib, ss, eb))

        def po_unit(ib, ss, eb):
            po = mip.tile([128, 512], F32, name=f"po{ib}{ss}{eb}", tag="mm")
            for cc in range(2):
                nc.tensor.matmul(
                    po,
                    lhsT=ctxn_sb[cc][ib][:, ss * 128 : (ss + 1) * 128],
                    rhs=wo_sb[:, cc, eb * 512 : (eb + 1) * 512],
                    start=(cc == 0),
                    stop=(cc == 1),
                )
            ot = work.tile([128, 512], BF16, name=f"ot{ib}{ss}{eb}", tag="ot")
            nc.vector.tensor_copy(out=ot, in_=po)
            row = ib * 512 + ss * 128
            nc.sync.dma_start(
                out=out_d[row : row + 128, eb * 512 : (eb + 1) * 512],
                in_=ot,
            )

        # ---- adaptive credit emission scheduler -------------------------
        elastic = deque()  # (cost_ns, emit_fn) in dependency-safe FIFO order
        rem = [0.0]

        def enq(cost, fn):
            elastic.append((cost, fn))
            rem[0] += cost

        pv_emitted = [-1]

        def run_unit():
            cost, fn = elastic.popleft()
            rem[0] -= cost
            fn()
            return cost

        forced = {}

        def pin(slot, fn):
            forced.setdefault(slot, []).append(fn)

        sbslots = {  # (pair, which, sb) -> first of two consecutive slots
            (0, "k", 1): 1,
            (0, "k", 2): 3,
            (0, "k", 3): 5,
            (0, "q", 1): 7,
            (0, "q", 2): 9,
            (0, "q", 3): 11,
            (1, "k", 0): 40,
            (1, "k", 1): 42,
            (1, "k", 2): 44,
            (1, "k", 3): 46,
            (1, "q", 0): 48,
            (1, "q", 1): 50,
            (1, "q", 2): 52,
            (1, "q", 3): 54,
        }
        for (pr, which, sb), s0 in sbslots.items():
            dst, w, bn = (
                (qt_sb, wq_sb, "bq") if which == "q" else (kt_sb, wk_sb, "bk")
            )
            for half in range(2):
                pin(
                    s0 + half,
                    lambda dst=dst, w=w, bn=bn, pr=pr, sb=sb, half=half: qk_half(
                        dst, w, bn, pr, sb, half
                    ),
                )

        # front: k0/q0 first s-block, paced by the DMA stream
        for half in range(2):
            qk_half(kt_sb, wk_sb, "bk", 0, 0, half)
        for half in range(2):
            qk_half(qt_sb, wq_sb, "bq", 0, 0, half)

        credit = 0.0
        for n in range(128):
            pair, ib, jj = n // 64, (n // 16) % 4, n % 16
            for fn in forced.pop(n, ()):
                fn()
            # deadlock guard: exp[n] reuses ex_ring slot n-EXBUFS, whose PV
            # consumer must already be in the engine queues. In the final
            # block keep the PV lag tiny so the tail is just the last
            # epilogue + out-proj chain.
            max_lag = 2 if n >= 112 else EXBUFS - 2
            while pv_emitted[0] < n - max_lag and elastic:
                run_unit()
            floor = 1300.0 if n >= 64 else 700.0
            budget = max(floor, min(2600.0, rem[0] / (128 - n) * 1.15))
            credit = min(credit + budget, 2.2 * budget)
            while elastic and credit >= elastic[0][0]:
                credit -= run_unit()
            sc_exp(pair, ib, jj, n)
            if pair == 0 and ib == 0:
                enq(1100, lambda jj=jj: v_unit(jj))
            enq(900, lambda pair=pair, ib=ib, jj=jj, n=n: (
                pv_unit(pair, ib, jj, n),
                pv_emitted.__setitem__(0, n),
            ))
            if jj == 15:
                enq(400, lambda pair=pair, ib=ib: epi(pair, ib))
                enq(600, lambda pair=pair, ib=ib: part2(pair, ib))
        while elastic:
            run_unit()

    nc.compile()
    return nc


def _get_graph(with_qkv_bias: bool):
    key = ("nc", with_qkv_bias)
    if key not in _CACHE:
        _CACHE[key] = _build_graph(with_qkv_bias)
    return _CACHE[key]


def _pack_w(w, nchunk, width):
    # [nchunk*128, width] -> [128, nchunk*width] e-major packing
    return np.ascontiguousarray(
        w.reshape(nchunk, 128, width).transpose(1, 0, 2).reshape(128, -1)
    )


def make_in_maps(x, Wq, bq, Wk, bk, Wv, bv, Wo, with_qkv_bias):
    bf16 = ml_dtypes.bfloat16
    in_maps = []
    for c in range(NCORES):
        b, g = c // GROUPS, c % GROUPS
        hs = slice(g * DH, (g + 1) * DH)
        m = {
            "xt": np.ascontiguousarray(x[b].T.astype(bf16)),
            "wq": _pack_w(Wq[hs, :].T.astype(bf16), 8, DH),
            "wk": _pack_w(Wk[hs, :].T.astype(bf16), 8, DH),
            "wv": _pack_w(Wv[hs, :].T.astype(bf16), 8, DH),
            "wo": _pack_w(Wo[:, hs].T.astype(bf16), 2, HID),
        }
        if with_qkv_bias:
            m["bq"] = np.ascontiguousarray(bq[None, hs].astype(bf16))
            m["bk"] = np.ascontiguousarray(bk[None, hs].astype(bf16))
            m["bv"] = np.ascontiguousarray(bv[None, hs].astype(bf16))
        in_maps.append(m)
    return in_maps


def kernel(x, Wq, bq, Wk, bk, Wv, bv, Wo, bo):
    global last_exec_time_ns, last_results
    from concourse.bass_utils import run_bass_kernel_spmd

    x = np.asarray(x, np.float32)
    Wq = np.asarray(Wq, np.float32)
    Wk = np.asarray(Wk, np.float32)
    Wv = np.asarray(Wv, np.float32)
    Wo = np.asarray(Wo, np.float32)
    bq = np.asarray(bq, np.float32)
    bk = np.asarray(bk, np.float32)
    bv = np.asarray(bv, np.float32)
    bo = np.asarray(bo, np.float32)

    with_qkv_bias = bool(np.any(bq) or np.any(bk) or np.any(bv))
    nc = _get_graph(with_qkv_bias)
    in_maps = make_in_maps(x, Wq, bq, Wk, bk, Wv, bv, Wo, with_qkv_bias)

    trace = os.environ.get("BASS_KERNEL_TRACE", "0") == "1"
    tdir = os.environ.get("BASS_KERNEL_TRACE_DIR") or None
    res = run_bass_kernel_spmd(
        nc, in_maps, list(range(NCORES)), trace=trace, tmpdir=tdir
    )
    last_exec_time_ns = res.exec_time_ns
    last_results = res

    out = np.zeros((B, S, HID), np.float32)
    for c in range(NCORES):
        out[c // GROUPS] += np.asarray(res.results[c]["out"], np.float32)
    out += bo
    return out


# revision 7
# speedup vs baseline: 1.0213x; 1.0213x over previous
"""Multi-head attention (B=2, S=2048, H=1024, 16 heads) on 8 TRN2 NeuronCores.

Sharding: core c -> batch b = c//4, head-group g = c%4 (heads 4g..4g+3).
Each core computes q/k/v projections for its 4 heads (tensor parallel),
full attention for those heads, and a partial output projection
(contribution of its 256 hidden dims). Host sums the 4 partials per batch
(bf16 partials, fp32 sum) and adds the output bias.

The kernel is jointly ACT(exp)- and PE-limited (~150-165us each). The
emission is built around a continuous scores->exp stream:

  - Weights are host-packed so each tensor is ONE contiguous DMA (DMA issue
    costs ~650ns each, serialized per queue). xt is DMA'd in column halves:
    cols 0:1024 on the Sync HWDGE queue, cols 1024:2048 on the Scalar HWDGE
    queue (idle until the exp stream starts), so the first k/q projection
    groups can finish ~8us in.
  - One "slot" per (pair, ib, jj): emits the two row-packed K=64 score
    matmuls (concurrent on the PE via row groups, start delta ~4ns) + the
    exp ACTIVATE, then pumps a work queue of deferrable units: v-proj
    chunks, qk projection half-groups (N=256, atomic psum lifetime), PV
    matmuls (lagging the exp stream), softmax epilogues and the output
    projection. Deadline-critical kt/qt projections are pinned to fixed
    slots. The pump budget adapts to the remaining queue so work cannot
    pile up into a serial tail; a force-drain rule keeps the PV lag under
    the ex ring depth so the statically-ordered engine queues cannot
    deadlock.
  - softmax epilogue: row-sum l comes free from a ones-column appended to V
    (PV matmul M=65); 1/l = exp(-ln(l)) on ACT, batched per (pair, ib) as
    one [1,1024] ln + one [1,1024] exp; broadcast to 64 partitions via a
    K=1 matmul; normalize on DVE. (reciprocal_approx_fast and gpsimd
    partition_broadcast both return garbage on this environment's HW --
    custom-DVE/gpsimd tables are not loaded -- so only ACT-table math and
    plain matmul/DVE ops are used.)
  - out-proj psum is CAST to bf16 and DMA'd as bf16 partials (halves the
    output traffic; host sums in fp32).
"""

import os
from collections import deque
from contextlib import ExitStack

import numpy as np
import ml_dtypes

B = 2
S = 2048
HID = 1024
NHEAD = 16
HDIM = 64
NCORES = 8
GROUPS = 4  # head-groups per batch (cores per batch)
DH = 256  # hidden dims per core (4 heads x 64)
SCALE = 1.0 / np.sqrt(np.float32(HDIM))  # 0.125

EXBUFS = 14  # ex ring depth; PV may lag the exp stream by at most EXBUFS-2

_CACHE = {}
last_exec_time_ns = None
last_results = None


def _build_graph(with_qkv_bias: bool):
    import concourse.bass as bass
    import concourse.mybir as mybir
    import concourse.tile as tile
    from concourse import bacc

    F32 = mybir.dt.float32
    BF16 = mybir.dt.bfloat16
    EXP = mybir.ActivationFunctionType.Exp
    LN = mybir.ActivationFunctionType.Ln

    # The kernel uses both Exp and Ln. Left alone, the act-table-load pass
    # alternates between exp_and_others and natural_log (17 loads, ~2.7us
    # each). Steer it to the one set containing both by hiding Exp/Ln from
    # every other set (indices must stay stable, so entries are kept).
    if not getattr(bacc, "_mha_act_tabs_patched", False):
        orig_gat = bacc.get_activation_tables

        def _gat(arch, _orig=orig_gat):
            out = {}
            for n, s in _orig(arch).items():
                if n != "natural_log_exp_and_others":
                    s = s - {EXP, LN}
                out[n] = s
            return out

        bacc.get_activation_tables = _gat
        bacc._mha_act_tabs_patched = True

    nc = bacc.Bacc()
    # weights host-packed e-major so each is one contiguous DMA
    xt_d = nc.declare_dram_parameter("xt", [HID, S], BF16, isOutput=False)
    wq_d = nc.declare_dram_parameter("wq", [128, 8 * DH], BF16, isOutput=False)
    wk_d = nc.declare_dram_parameter("wk", [128, 8 * DH], BF16, isOutput=False)
    wv_d = nc.declare_dram_parameter("wv", [128, 8 * DH], BF16, isOutput=False)
    wo_d = nc.declare_dram_parameter("wo", [128, 2 * HID], BF16, isOutput=False)
    if with_qkv_bias:
        bq_d = nc.declare_dram_parameter("bq", [1, DH], BF16, isOutput=False)
        bk_d = nc.declare_dram_parameter("bk", [1, DH], BF16, isOutput=False)
        bv_d = nc.declare_dram_parameter("bv", [1, DH], BF16, isOutput=False)
    out_d = nc.declare_dram_parameter("out", [S, HID], BF16, isOutput=True)

    with ExitStack() as ctx:
        tc = ctx.enter_context(tile.TileContext(nc))
        cons = ctx.enter_context(tc.tile_pool(name="cons", bufs=1))
        work = ctx.enter_context(tc.tile_pool(name="work", bufs=3))
        scp = ctx.enter_context(tc.tile_pool(name="scp", bufs=2, space="PSUM"))
        pvp = ctx.enter_context(tc.tile_pool(name="pvp", bufs=1, space="PSUM"))
        mip = ctx.enter_context(tc.tile_pool(name="mip", bufs=2, space="PSUM"))

        # ---- SBUF tiles -------------------------------------------------
        xt_sb = [
            cons.tile([128, S], BF16, name=f"xt{e}", tag=f"xt{e}") for e in range(8)
        ]
        wq_sb = cons.tile([128, 8, DH], BF16, name="wq", tag="wq")
        wk_sb = cons.tile([128, 8, DH], BF16, name="wk", tag="wk")
        wv_sb = cons.tile([128, 8, DH], BF16, name="wv", tag="wv")
        wo_sb = cons.tile([128, 2, HID], BF16, name="wo", tag="wo")

        # Input DMA is bandwidth-bound (~130 GB/s aggregate, shared DRAM
        # channel), so the schedule delivers bytes in exactly the order the
        # exp stream consumes them, split across the two HWDGE queues.
        # Quarter-blocks (cols 256 wide) for the first two kt/qt s-blocks,
        # halves after:
        #   Sync:   wk, xtq0 (cols 0:256), xtq3 (768:1024), xt 1024:1536, wo
        #   Scalar: wq, xtq1 (256:512), xtq2 (512:768), wv, xt 1536:2048
        def dma_xt(eng, e, c0, c1):
            eng.dma_start(
                out=xt_sb[e][:, c0:c1],
                in_=xt_d[e * 128 : (e + 1) * 128, c0:c1],
            )

        nc.sync.dma_start(out=wk_sb, in_=wk_d[:, :])
        nc.scalar.dma_start(out=wq_sb, in_=wq_d[:, :])
        for e in range(8):
            dma_xt(nc.sync, e, 0, 256)
        for e in range(8):
            dma_xt(nc.scalar, e, 256, 512)
        for e in range(8):
            dma_xt(nc.sync, e, 768, 1024)
        for e in range(8):
            dma_xt(nc.scalar, e, 512, 768)
        nc.scalar.dma_start(out=wv_sb, in_=wv_d[:, :])
        for e in range(8):
            dma_xt(nc.sync, e, 1024, 1536)
        for e in range(8):
            dma_xt(nc.scalar, e, 1536, 2048)
        nc.sync.dma_start(out=wo_sb, in_=wo_d[:, :])

        ones1 = cons.tile([1, 512], BF16, name="ones1", tag="ones1")
        nc.vector.memset(ones1, 1.0)
        # ones row at partition 64: stationary operand of the K=1 broadcast
        # matmul replicating rl16 to 64 partitions.
        ones64 = cons.tile([65, 64], BF16, name="ones64", tag="ones64")
        nc.vector.memset(ones64[64:65, :], 1.0)

        if with_qkv_bias:
            bias_sb = {}
            for nm, d in (("bq", bq_d), ("bk", bk_d), ("bv", bv_d)):
                t = cons.tile([1, DH], BF16, name=f"{nm}s", tag=f"{nm}s")
                nc.sync.dma_start(out=t, in_=d[:, :])
                bias_sb[nm] = t

        qt_sb = [
            cons.tile([128, S], BF16, name=f"qt{c}", tag=f"qt{c}") for c in range(2)
        ]
        kt_sb = [
            cons.tile([128, S], BF16, name=f"kt{c}", tag=f"kt{c}") for c in range(2)
        ]
        v_sb = [
            cons.tile([128, 4, 65], BF16, name=f"v{j}", tag=f"v{j}") for j in range(16)
        ]
        ctxn_sb = [
            [
                cons.tile([128, 512], BF16, name=f"cx{c}_{i}", tag=f"cx{c}_{i}")
                for i in range(4)
            ]
            for c in range(2)
        ]

        # ---- unit emitters ----------------------------------------------
        def qk_half(dst_sb, w_sb, bias_nm, cc, sb, half):
            # half-width projection group: atomic psum lifetime (N=256)
            ps = mip.tile([128, 256], F32, name=f"pq{cc}{sb}{half}", tag="mm")
            c0 = sb * 512 + half * 256
            for e in range(8):
                nc.tensor.matmul(
                    ps,
                    lhsT=w_sb[:, e, cc * 128 : (cc + 1) * 128],
                    rhs=xt_sb[e][:, c0 : c0 + 256],
                    start=(e == 0),
                    stop=(e == 7 and not with_qkv_bias),
                )
            if with_qkv_bias:
                nc.tensor.matmul(
                    ps,
                    lhsT=bias_sb[bias_nm][:, cc * 128 : (cc + 1) * 128],
                    rhs=ones1[:, 0:256],
                    start=False,
                    stop=True,
                )
            nc.vector.tensor_copy(out=dst_sb[cc][:, c0 : c0 + 256], in_=ps)

        def v_unit(jj):
            # v [s, d] natural, stored per j-chunk as [128, 4, 65] with a
            # ones column at [:, :, 64] for the softmax row-sum.
            ps = mip.tile([128, DH], F32, name=f"pv{jj}", tag="mm")
            for e in range(8):
                nc.tensor.matmul(
                    ps,
                    lhsT=xt_sb[e][:, jj * 128 : (jj + 1) * 128],
                    rhs=wv_sb[:, e, :],
                    start=(e == 0),
                    stop=(e == 7 and not with_qkv_bias),
                )
            if with_qkv_bias:
                nc.tensor.matmul(
                    ps,
                    lhsT=ones1[:, 0:128],
                    rhs=bias_sb["bv"],
                    start=False,
                    stop=True,
                )
            nc.vector.tensor_copy(
                out=v_sb[jj][:, :, 0:64],
                in_=ps.rearrange("p (h d) -> p h d", h=4),
            )
            nc.vector.memset(v_sb[jj][:, :, 64:65], 1.0)

        ex_ring = {}  # attention-stream index -> ex tile
        pv_ps = {}  # (pair, ib) -> [psum tiles h0, h1]
        epi_sb = {}  # (pair, ib) -> (pvsw, rl16)

        def sc_exp(pair, ib, jj, n):
            ps = scp.tile([128, 1024], F32, name=f"sc{pair}{ib}{jj}", tag="sc")
            for h in range(2):
                nc.tensor.matmul(
                    ps[:, h * 512 : (h + 1) * 512],
                    lhsT=kt_sb[pair][
                        h * 64 : (h + 1) * 64, jj * 128 : (jj + 1) * 128
                    ],
                    rhs=qt_sb[pair][
                        h * 64 : (h + 1) * 64, ib * 512 : (ib + 1) * 512
                    ],
                    start=True,
                    stop=True,
                )
            ex = work.tile(
                [128, 1024], BF16, name=f"ex{n}", tag="ex", bufs=EXBUFS
            )
            nc.scalar.activation(out=ex, in_=ps, func=EXP, scale=float(SCALE))
            ex_ring[n] = ex

        def pv_unit(pair, ib, jj, n):
            if jj == 0:
                pv_ps[(pair, ib)] = [
                    pvp.tile([65, 512], F32, name=f"pva{pair}{ib}", tag="pva"),
                    pvp.tile([65, 512], F32, name=f"pvb{pair}{ib}", tag="pvb"),
                ]
            ex = ex_ring.pop(n)
            for h in range(2):
                nc.tensor.matmul(
                    pv_ps[(pair, ib)][h],
                    lhsT=v_sb[jj][:, pair * 2 + h, :],
                    rhs=ex[:, h * 512 : (h + 1) * 512],
                    start=(jj == 0),
                    stop=(jj == 15),
                )

        def epi(pair, ib):
            # psum -> sbuf copies free the PV banks; one wide ln + one wide
            # exp on ACT give rl16 = 1/l for both heads in 2.3us.
            pvsw = work.tile(
                [65, 1024], F32, name=f"ps{pair}{ib}", tag="pvs", bufs=4
            )
            for h in range(2):
                nc.vector.tensor_copy(
                    out=pvsw[:, h * 512 : (h + 1) * 512],
                    in_=pv_ps[(pair, ib)][h],
                )
            lnl = work.tile([65, 1024], F32, name=f"ln{pair}{ib}", tag="lnl")
            nc.scalar.activation(out=lnl[64:65, :], in_=pvsw[64:65, :], func=LN)
            rl16 = work.tile(
                [65, 1024], BF16, name=f"rl{pair}{ib}", tag="rl", bufs=4
            )
            nc.scalar.activation(
                out=rl16[64:65, :], in_=lnl[64:65, :], func=EXP, scale=-1.0
            )
            del pv_ps[(pair, ib)]
            epi_sb[(pair, ib)] = (pvsw, rl16)

        def part2(pair, ib):
            pvsw, rl16 = epi_sb.pop((pair, ib))
            for h in range(2):
                bc = mip.tile([64, 512], F32, name=f"bc{pair}{ib}{h}", tag="mm")
                nc.tensor.matmul(
                    bc,
                    lhsT=ones64[64:65, :],
                    rhs=rl16[64:65, h * 512 : (h + 1) * 512],
                    start=True,
                    stop=True,
                )
                # DVE may read only one PSUM operand: in0 SBUF, in1 PSUM.
                if h == 0:
                    nc.vector.tensor_mul(
                        out=ctxn_sb[pair][ib][0:64, :],
                        in0=pvsw[0:64, 0:512],
                        in1=bc,
                    )
                else:
                    tmp = work.tile(
                        [64, 512], BF16, name=f"tmp{pair}{ib}", tag="tmp"
                    )
                    nc.vector.tensor_mul(
                        out=tmp, in0=pvsw[0:64, 512:1024], in1=bc
                    )
                    nc.sync.dma_start(out=ctxn_sb[pair][ib][64:128, :], in_=tmp)
            if pair == 1:
                for ss in range(4):
                    for eb in range(2):
                        enq(700, lambda ss=ss, eb=eb: po_unit(ib, ss, eb))

        def po_unit(ib, ss, eb):
            po = mip.tile([128, 512], F32, name=f"po{ib}{ss}{eb}", tag="mm")
            for cc in range(2):
                nc.tensor.matmul(
                    po,
                    lhsT=ctxn_sb[cc][ib][:, ss * 128 : (ss + 1) * 128],
                    rhs=wo_sb[:, cc, eb * 512 : (eb + 1) * 512],
                    start=(cc == 0),
                    stop=(cc == 1),
                )
            ot = work.tile([128, 512], BF16, name=f"ot{ib}{ss}{eb}", tag="ot")
            nc.vector.tensor_copy(out=ot, in_=po)
            row = ib * 512 + ss * 128
            nc.sync.dma_start(
                out=out_d[row : row + 128, eb * 512 : (eb + 1) * 512],
                in_=ot,
            )

        # ---- adaptive credit emission scheduler -------------------------
        elastic = deque()  # (cost_ns, emit_fn) in dependency-safe FIFO order
        rem = [0.0]

        def enq(cost, fn):
            elastic.append((cost, fn))
            rem[0] += cost

        pv_emitted = [-1]

        def run_unit():
            cost, fn = elastic.popleft()
            rem[0] -= cost
            fn()
            return cost

        forced = {}

        def pin(slot, fn):
            forced.setdefault(slot, []).append(fn)

        sbslots = {  # (pair, which, sb) -> first of two consecutive slots
            (0, "k", 1): 1,
            (0, "k", 2): 3,
            (0, "k", 3): 5,
            (0, "q", 1): 7,
            (0, "q", 2): 9,
            (0, "q", 3): 11,
            (1, "k", 0): 40,
            (1, "k", 1): 42,
            (1, "k", 2): 44,
            (1, "k", 3): 46,
            (1, "q", 0): 48,
            (1, "q", 1): 50,
            (1, "q", 2): 52,
            (1, "q", 3): 54,
        }
        for (pr, which, sb), s0 in sbslots.items():
            dst, w, bn = (
                (qt_sb, wq_sb, "bq") if which == "q" else (kt_sb, wk_sb, "bk")
            )
            for half in range(2):
                pin(
                    s0 + half,
                    lambda dst=dst, w=w, bn=bn, pr=pr, sb=sb, half=half: qk_half(
                        dst, w, bn, pr, sb, half
                    ),
                )

        # front: k0/q0 first s-block, paced by the DMA stream
        for half in range(2):
            qk_half(kt_sb, wk_sb, "bk", 0, 0, half)
        for half in range(2):
            qk_half(qt_sb, wq_sb, "bq", 0, 0, half)

        credit = 0.0
        for n in range(128):
            pair, ib, jj = n // 64, (n // 16) % 4, n % 16
            for fn in forced.pop(n, ()):
                fn()
            # deadlock guard: exp[n] reuses ex_ring slot n-EXBUFS, whose PV
            # consumer must already be in the engine queues. In the final
            # block keep the PV lag tiny so the tail is just the last
            # epilogue + out-proj chain.
            max_lag = 2 if n >= 112 else EXBUFS - 2
            while pv_emitted[0] < n - max_lag and elastic:
                run_unit()
            budget = max(700.0, min(2600.0, rem[0] / (128 - n) * 1.15))
            credit = min(credit + budget, 2.2 * budget)
            while elastic and credit >= elastic[0][0]:
                credit -= run_unit()
            sc_exp(pair, ib, jj, n)
            if pair == 0 and ib == 0:
                enq(1100, lambda jj=jj: v_unit(jj))
            enq(900, lambda pair=pair, ib=ib, jj=jj, n=n: (
                pv_unit(pair, ib, jj, n),
                pv_emitted.__setitem__(0, n),
            ))
            if jj == 15:
                enq(400, lambda pair=pair, ib=ib: epi(pair, ib))
                enq(600, lambda pair=pair, ib=ib: part2(pair, ib))
        while elastic:
            run_unit()

    nc.compile()
    return nc


def _get_graph(with_qkv_bias: bool):
    key = ("nc", with_qkv_bias)
    if key not in _CACHE:
        _CACHE[key] = _build_graph(with_qkv_bias)
    return _CACHE[key]


def _pack_w(w, nchunk, width):
    # [nchunk*128, width] -> [128, nchunk*width] e-major packing
    return np.ascontiguousarray(
        w.reshape(nchunk, 128, width).transpose(1, 0, 2).reshape(128, -1)
    )


def make_in_maps(x, Wq, bq, Wk, bk, Wv, bv, Wo, with_qkv_bias):
    bf16 = ml_dtypes.bfloat16
    in_maps = []
    for c in range(NCORES):
        b, g = c // GROUPS, c % GROUPS
        hs = slice(g * DH, (g + 1) * DH)
        m = {
            "xt": np.ascontiguousarray(x[b].T.astype(bf16)),
            "wq": _pack_w(Wq[hs, :].T.astype(bf16), 8, DH),
            "wk": _pack_w(Wk[hs, :].T.astype(bf16), 8, DH),
            "wv": _pack_w(Wv[hs, :].T.astype(bf16), 8, DH),
            "wo": _pack_w(Wo[:, hs].T.astype(bf16), 2, HID),
        }
        if with_qkv_bias:
            m["bq"] = np.ascontiguousarray(bq[None, hs].astype(bf16))
            m["bk"] = np.ascontiguousarray(bk[None, hs].astype(bf16))
            m["bv"] = np.ascontiguousarray(bv[None, hs].astype(bf16))
        in_maps.append(m)
    return in_maps


def kernel(x, Wq, bq, Wk, bk, Wv, bv, Wo, bo):
    global last_exec_time_ns, last_results
    from concourse.bass_utils import run_bass_kernel_spmd

    x = np.asarray(x, np.float32)
    Wq = np.asarray(Wq, np.float32)
    Wk = np.asarray(Wk, np.float32)
    Wv = np.asarray(Wv, np.float32)
    Wo = np.asarray(Wo, np.float32)
    bq = np.asarray(bq, np.float32)
    bk = np.asarray(bk, np.float32)
    bv = np.asarray(bv, np.float32)
    bo = np.asarray(bo, np.float32)

    with_qkv_bias = bool(np.any(bq) or np.any(bk) or np.any(bv))
    nc = _get_graph(with_qkv_bias)
    in_maps = make_in_maps(x, Wq, bq, Wk, bk, Wv, bv, Wo, with_qkv_bias)

    trace = os.environ.get("BASS_KERNEL_TRACE", "0") == "1"
    tdir = os.environ.get("BASS_KERNEL_TRACE_DIR") or None
    res = run_bass_kernel_spmd(
        nc, in_maps, list(range(NCORES)), trace=trace, tmpdir=tdir
    )
    last_exec_time_ns = res.exec_time_ns
    last_results = res

    out = np.zeros((B, S, HID), np.float32)
    for c in range(NCORES):
        out[c // GROUPS] += np.asarray(res.results[c]["out"], np.float32)
    out += bo
    return out
